# revision 1
# baseline (speedup 1.0000x reference)
"""Non-local (dot-product attention) block kernel for Trainium2, 8 cores.

Reference math (per sample):
    t = theta_w @ xf + theta_b           (D, N)
    p = (phi_w @ xf + phi_b) / N         (D, N)
    g = g_w @ xf + g_b                   (D, N)
    f = t.T p  (NxN attention);  y = f g.T;  z = BN(w_w y) + x

Algebraic collapse (matmul associativity, BN folded on host):
    M[e,d] = sum_m p[e,m] g[d,m]                       (D x D)
    V[c,e] = sum_d w'[c,d] M[e,d]       w' = diag(inv) w_w
    z      = V @ t + b' 1^T + x
so the N x N attention matrix and y never exist -- per sample two
projection passes over x (t and [phi|g]) plus a D x D contraction, then
one (C x D) @ (D x N) output matmul.

Sharding: data-parallel over batch B=8, one sample per NeuronCore, no
collectives. Matmul inputs bf16 (fp32 PSUM accumulation), biases and
residual applied in fp32; output fp32.

HW notes baked in:
  - One byte-packed weight DMA (each dma_start has ~600ns fixed cost, and
    in-DMA bandwidth is a shared ~320GB/s pool -> strict need-order:
    weights, X0, X1).
  - Dummy matmuls over a zeroed tile warm the PE HAM clock gate (1.2 ->
    2.4 GHz takes ~3.4us of sustained activity) while x is still in
    flight, and X0-only (k=0) work is emitted ahead of X1-dependent work.
  - theta projection runs interleaved with the phi|g projection (ACT does
    its PSUM->SBUF copies; DVE does the phi|g bias adds), so after the
    tiny M -> V chain the output phase is just 12 K=128 matmuls racing
    the output DMAs.
  - Output DMAs alternate the two HWDGE rings, (128, 1024) fp32 chunks
    (4KB descriptors).
"""

import numpy as np

B, C, HH, WW = 8, 256, 96, 32
N = HH * WW          # 3072
D = 128              # inter_channels
BN_EPS = 1e-5
NT = N // 128        # 24 pixel chunks
NR = N // 1024       # 3 pixel regions
N_CORES = 8

_NC = None


def _build_nc():
    from contextlib import ExitStack

    import concourse.bass as bass
    import concourse.bacc as bacc
    import concourse.tile as tile
    from concourse import mybir

    f32 = mybir.dt.float32
    bf16 = mybir.dt.bfloat16
    AF = mybir.ActivationFunctionType
    ALU = mybir.AluOpType

    nc = bacc.Bacc(
        "TRN2",
        target_bir_lowering=False,
        debug=False,
        num_devices=N_CORES,
    )

    x = nc.dram_tensor("x", [C, N], bf16, kind="ExternalInput").ap()
    # wpk byte-packs, per partition row: aux 260 f32 | pgW 512 bf16 |
    # thw 260 bf16 | wT 256 bf16  => 774 f32 columns total
    wpk = nc.dram_tensor("wpk", [128, 774], f32, kind="ExternalInput").ap()
    out = nc.dram_tensor("out", [C, N], f32, kind="ExternalOutput").ap()

    with tile.TileContext(nc) as tc, ExitStack() as ctx:
        const = ctx.enter_context(tc.tile_pool(name="const", bufs=1))
        zpool = ctx.enter_context(tc.tile_pool(name="zpool", bufs=3))
        ps_mm = ctx.enter_context(tc.tile_pool(name="ps_mm", bufs=3, space="PSUM"))
        ps_sm = ctx.enter_context(tc.tile_pool(name="ps_sm", bufs=1, space="PSUM"))

        X0 = const.tile([128, N], bf16)
        X1 = const.tile([128, N], bf16)
        t_sb = const.tile([128, N], bf16)
        pg_sb = const.tile([128, NT * 256], bf16)
        m2_sb = const.tile([128, 128], bf16)
        w2_sb = const.tile([128, 256], bf16)
        wz = const.tile([128, 512], bf16)
        wpk_sb = const.tile([128, 774], f32)

        nc.sync.dma_start(out=wpk_sb, in_=wpk)
        nc.sync.dma_start(out=X0, in_=x[0:128, :])
        nc.sync.dma_start(out=X1, in_=x[128:256, :])

        aux_sb = wpk_sb[:, 0:260]
        pgW = wpk_sb[:, 260:516].bitcast(bf16)     # (128, 512)
        thw_sb = wpk_sb[:, 516:646].bitcast(bf16)  # (128, 260)
        wT = wpk_sb[:, 646:774].bitcast(bf16)      # (128, 256)

        b_out = [aux_sb[:, 0:1], aux_sb[:, 1:2]]
        theta_b = aux_sb[:, 2:3]
        _pgb = aux_sb[:, 4:260]
        pg_bias4 = bass.AP(
            tensor=_pgb.tensor, offset=_pgb.offset,
            ap=[list(_pgb.ap[0]), [0, 4], list(_pgb.ap[1])],
        )

        # PE warm-up on a zeroed tile: the HAM clock gate needs ~3.4us of
        # sustained activity to lift the PE 1.2 -> 2.4 GHz; burn the x-DMA
        # wait so real matmuls run warm from their first instruction.
        nc.vector.memset(wz, 0.0)
        wup = ps_mm.tile([128, 512], f32, tag="mm", name="wup")
        for _ in range(20):
            nc.tensor.matmul(
                wup, lhsT=wz[:, 0:128], rhs=wz, start=True, stop=True
            )

        # m2[d,e] = sum_m g[m,d] p[m,e] accumulates across the whole pg phase
        pm = ps_sm.tile([128, 128], f32, tag="sm")

        # interleaved theta + phi|g projections.
        #   t group: (128, 512) of t in (D, N), ACT copy w/ theta_b bias
        #   pg group: 4 pixel chunks, DVE bias add via broadcast view,
        #             then 4 m2 accumulation matmuls
        for grp in range(6):
            fsl = slice(grp * 512, (grp + 1) * 512)
            pt = ps_mm.tile([128, 1024], f32, tag="mm", name=f"pt{grp}")
            nc.tensor.matmul(
                pt[:, 0:512], lhsT=thw_sb[:, 0:128], rhs=X0[:, fsl],
                start=True, stop=False,
            )
            nc.tensor.matmul(
                pt[:, 0:512], lhsT=thw_sb[:, 128:256], rhs=X1[:, fsl],
                start=False, stop=True,
            )
            nc.scalar.activation(
                out=t_sb[:, fsl], in_=pt[:, 0:512], func=AF.Identity,
                bias=theta_b, scale=1.0,
            )

            pp = ps_mm.tile([128, 1024], f32, tag="mm", name=f"pp{grp}")
            for i in range(4):
                nt = grp * 4 + i
                nsl = slice(nt * 128, (nt + 1) * 128)
                psl = slice(i * 256, (i + 1) * 256)
                nc.tensor.matmul(
                    pp[:, psl], lhsT=X0[:, nsl], rhs=pgW[:, 0:256],
                    start=True, stop=False,
                )
                nc.tensor.matmul(
                    pp[:, psl], lhsT=X1[:, nsl], rhs=pgW[:, 256:512],
                    start=False, stop=True,
                )
            gsl = slice(grp * 1024, (grp + 1) * 1024)
            nc.vector.tensor_add(
                pg_sb[:, gsl].rearrange("p (a b) -> p a b", a=4),
                pp.rearrange("p (a b) -> p a b", a=4),
                pg_bias4,
            )
            for i in range(4):
                nt = grp * 4 + i
                nc.tensor.matmul(
                    pm,
                    lhsT=pg_sb[:, nt * 256 + 128 : (nt + 1) * 256],
                    rhs=pg_sb[:, nt * 256 : nt * 256 + 128],
                    start=(nt == 0),
                    stop=(nt == NT - 1),
                )
        nc.scalar.copy(out=m2_sb, in_=pm)

        # w2[e,c] = sum_d m2[d,e] w'[c,d]  (= V[c,e])
        pw = ps_sm.tile([128, 256], f32, tag="sm")
        nc.tensor.matmul(pw, lhsT=m2_sb, rhs=wT, start=True, stop=True)
        nc.scalar.copy(out=w2_sb, in_=pw)

        # z[c,n] = sum_e w2[e,c] t[e,n] + b'[c] + x[c,n]
        ndma = 0
        for j in range(NR):
            for cc in range(2):
                jsl = slice(j * 1024, (j + 1) * 1024)
                pz = ps_mm.tile([128, 1024], f32, tag="mm")
                for f in range(2):
                    fsl = slice(j * 1024 + f * 512, j * 1024 + (f + 1) * 512)
                    nc.tensor.matmul(
                        pz[:, f * 512 : (f + 1) * 512],
                        lhsT=w2_sb[:, cc * 128 : (cc + 1) * 128],
                        rhs=t_sb[:, fsl], start=True, stop=True,
                    )
                xres = (X0 if cc == 0 else X1)[:, jsl]
                z_sb = zpool.tile([128, 1024], f32, tag="z_sb")
                nc.vector.scalar_tensor_tensor(
                    out=z_sb, in0=pz, scalar=b_out[cc],
                    in1=xres, op0=ALU.add, op1=ALU.add,
                )
                eng = nc.sync if ndma % 2 == 0 else nc.scalar
                ndma += 1
                eng.dma_start(
                    out=out[cc * 128 : (cc + 1) * 128, jsl], in_=z_sb,
                )

    nc.compile()
    return nc


def _get_nc():
    global _NC
    if _NC is None:
        _NC = _build_nc()
    return _NC


# test.py reads this after a traced run to get exec_time_ns
last_results = None


def _prep_inputs(inputs):
    import ml_dtypes

    bf16 = ml_dtypes.bfloat16

    x = np.asarray(inputs["x"], dtype=np.float32)
    theta_w = np.asarray(inputs["theta_w"], np.float32)
    theta_b = np.asarray(inputs["theta_b"], np.float32)
    phi_w = np.asarray(inputs["phi_w"], np.float32)
    phi_b = np.asarray(inputs["phi_b"], np.float32)
    g_w = np.asarray(inputs["g_w"], np.float32)
    g_b = np.asarray(inputs["g_b"], np.float32)
    w_w = np.asarray(inputs["w_w"], np.float32)
    w_b = np.asarray(inputs["w_b"], np.float32)
    bn_gamma = np.asarray(inputs["bn_gamma"], np.float32)
    bn_beta = np.asarray(inputs["bn_beta"], np.float32)
    bn_mean = np.asarray(inputs["bn_mean"], np.float32)
    bn_var = np.asarray(inputs["bn_var"], np.float32)

    inv = bn_gamma / np.sqrt(bn_var + BN_EPS)
    b_out = (w_b - bn_mean) * inv + bn_beta                   # (C,)

    aux = np.zeros((128, 260), np.float32)
    aux[:, 0] = b_out[:128]
    aux[:, 1] = b_out[128:]
    aux[:, 2] = theta_b
    aux[:, 4:260] = np.concatenate([phi_b / N, g_b])[None, :]

    pgw = np.concatenate([phi_w.T / N, g_w.T], axis=1)        # (C, 2D)
    pgw_pk = np.concatenate([pgw[0:128], pgw[128:256]], axis=1)  # (128, 512)
    # thw holds theta_w^T packed as [c-chunk0 | c-chunk1]: lhsT for the t
    # projection needs (c-part, d-free) = theta_w.T chunks
    thwT = theta_w.T                                          # (C, D)
    thw = np.zeros((D, 260), np.float32)
    thw[:, 0:128] = thwT[0:128, :]
    thw[:, 128:256] = thwT[128:256, :]
    wwt = (w_w * inv[:, None]).T                              # (D, C)

    wpk_u8 = np.concatenate(
        [
            aux.view(np.uint8),                               # 1040 B
            np.ascontiguousarray(pgw_pk).astype(bf16).view(np.uint8),  # 1024 B
            np.ascontiguousarray(thw).astype(bf16).view(np.uint8),     # 520 B
            np.ascontiguousarray(wwt).astype(bf16).view(np.uint8),     # 512 B
        ],
        axis=1,
    )
    assert wpk_u8.shape == (128, 3096), wpk_u8.shape
    wpk = np.ascontiguousarray(wpk_u8).view(np.float32)       # (128, 774)

    xf = x.reshape(B, C, N).astype(bf16)
    return xf, {"wpk": wpk}


def kernel(**inputs):
    from concourse.bass_utils import run_bass_kernel_spmd

    global last_results

    xf, shared = _prep_inputs(inputs)
    in_maps = [dict(shared, x=np.ascontiguousarray(xf[b])) for b in range(B)]

    nc = _get_nc()
    res = run_bass_kernel_spmd(nc, in_maps, list(range(N_CORES)))
    last_results = res

    z = np.stack([res.results[b]["out"] for b in range(B)])
    return z.reshape(B, C, HH, WW).astype(np.float32)



# revision 10
# speedup vs baseline: 1.0390x; 1.0390x over previous
"""Non-local (dot-product attention) block kernel for Trainium2, 8 cores.

Reference math (per sample):
    t = theta_w @ X + bt 1^T            (D, N)
    p = phi_w @ X + bp 1^T              (D, N)
    g = (g_w @ X + bg 1^T)^T            (N, D)
    f = t^T p / N;  y = f g;  z = BN(w_w y^T) + x

Gram-form collapse (no softmax => pure matmul associativity). Since phi/g
only appear through M = p g, and t only through W2 = V theta_w:
    S  = X X^T                      (C, C)  Gram matrix
    sx = X 1                        (C,)    row sums (free via ones column)
    Mt = g_w S phi_w^T + bg(u+N bp)^T + v bp^T   u=phi_w sx, v=g_w sx
    R1 = Mt^T w''^T  (= V^T)        w'' = diag(inv) w_w / N
    T2 = theta_w^T R1 + I           (= W2^T + I: residual folded into weights)
    b2 = R1^T bt + b'
    z  = T2^T X + b2 1^T            one (C,C) x (C,N) output matmul

So the per-pixel work is just two C x C x N matmuls (Gram + output); the
whole projection/bias machinery shrinks to a ~10-matmul D-sized chain.

Device plan per core (data-parallel, one sample per core, no collectives):
  - Inputs: x uploaded twice (n-major xt for the Gram, c-major xc for the
    output matmul), fp16; weights byte-packed into wpk + a 1-partition aux.
  - S accumulates in PSUM over 24 pixel chunks streamed by DMA; a host-side
    ones column (257th) makes column 256 of S the row-sums sx.
  - Chain runs on tiny matmuls with ACT/DVE/Pool doing the PSUM->SBUF hops.
  - Output phase: 12 (128,512) matmul pairs; per-partition bias b2 applied
    during the PSUM->SBUF copy (rotated over ACT/DVE/Pool), fp16 out DMA
    issued by the same engine that assembled the tile; host widens to fp32.
"""

import numpy as np

B, C, HH, WW = 8, 256, 96, 32
N = HH * WW          # 3072
D = 128              # inter_channels
BN_EPS = 1e-5
NT = N // 128        # 24 pixel chunks
N_CORES = 8
NWARM = 6            # dummy matmuls to lift the PE HAM clock gate

_NC = None


def _build_nc():
    from contextlib import ExitStack

    import concourse.bass as bass
    import concourse.bacc as bacc
    import concourse.tile as tile
    from concourse import mybir

    f32 = mybir.dt.float32
    f16 = mybir.dt.float16
    AF = mybir.ActivationFunctionType

    nc = bacc.Bacc(
        "TRN2",
        target_bir_lowering=False,
        debug=False,
        num_devices=N_CORES,
    )

    # xt: 24 blocks of 257 cols: block i = x^T[128i:128(i+1), :] | ones
    xt = nc.dram_tensor("xt", [128, NT * 257], f16, kind="ExternalInput").ap()
    # xc: 6 blocks of 1024: block j = [x[0:128, 512j:512j+512] | x[128:256, ...]]
    xc = nc.dram_tensor("xc", [128, 6144], f16, kind="ExternalInput").ap()
    # wpk f16 cols: pgWT 512 | wT2 256 | thW 256 | Ipk 512 | bt 2 | pad 2
    wpk = nc.dram_tensor("wpk", [128, 770], f32, kind="ExternalInput").ap()
    # aux f16 cols: bg 128 | bp 128 | b'0 128 | b'1 128 ; f32 cols 256:384 = N*bp
    aux = nc.dram_tensor("aux", [1, 384], f32, kind="ExternalInput").ap()
    # out: [z[0:128, :] | z[128:256, :]] fp16
    out = nc.dram_tensor("out", [128, 6144], f16, kind="ExternalOutput").ap()

    with tile.TileContext(nc) as tc, ExitStack() as ctx:
        const = ctx.enter_context(tc.tile_pool(name="const", bufs=1))
        zpool = ctx.enter_context(tc.tile_pool(name="zpool", bufs=4))
        psS = ctx.enter_context(tc.tile_pool(name="psS", bufs=1, space="PSUM"))
        psC = ctx.enter_context(tc.tile_pool(name="psC", bufs=2, space="PSUM"))
        psZ = ctx.enter_context(tc.tile_pool(name="psZ", bufs=3, space="PSUM"))

        xt_sb = const.tile([128, NT * 257], f16)
        xc_sb = const.tile([128, 6144], f16)
        wpk_sb = const.tile([128, 770], f32)
        aux_sb = const.tile([1, 384], f32)
        S0_sb = const.tile([128, 257], f16)
        S1_sb = const.tile([128, 257], f16)
        uv_sb = const.tile([1, 256], f16)
        row0_sb = const.tile([1, 128], f16)
        SG_sb = const.tile([128, 256], f16)
        Mt_sb = const.tile([128, 128], f16)
        R1_sb = const.tile([128, 256], f16)
        T2_sb = const.tile([128, 512], f16)
        b2_sb = const.tile([128, 2], f32)
        wz = const.tile([128, 512], f16)

        wpk16 = wpk_sb.bitcast(f16)   # (128, 1540)
        aux16 = aux_sb.bitcast(f16)   # (1, 768)

        # weight/aux DMAs first, then xt pieces (needed now), then xc
        # (needed only from the output phase on).
        nc.sync.dma_start(out=wpk_sb, in_=wpk)
        nc.scalar.dma_start(out=aux_sb, in_=aux)
        for p in range(6):
            psl = slice(p * 4 * 257, (p + 1) * 4 * 257)
            nc.sync.dma_start(out=xt_sb[:, psl], in_=xt[:, psl])
        for q in range(3):
            qsl = slice(q * 2048, (q + 1) * 2048)
            nc.scalar.dma_start(out=xc_sb[:, qsl], in_=xc[:, qsl])

        # PE warm-up on a zeroed tile while the first xt piece is in flight
        # (HAM clock gate needs ~3.4us of sustained PE activity to lift
        # 1.2 -> 2.4 GHz; the S matmuls continue the streak).
        nc.vector.memset(wz, 0.0)
        for _ in range(NWARM):
            wup = psZ.tile([128, 512], f32, tag="z", name="wup")
            nc.tensor.matmul(wup, lhsT=wz[:, 0:128], rhs=wz, start=True, stop=True)

        # S = X X^T accumulated over 24 pixel chunks; col 256 = sx (ones col)
        S0 = psS.tile([128, 512], f32, tag="s0", name="S0")[:, 0:257]
        S1 = psS.tile([128, 512], f32, tag="s1", name="S1")[:, 0:257]
        for i in range(NT):
            base = i * 257
            nc.tensor.matmul(
                S0, lhsT=xt_sb[:, base : base + 128],
                rhs=xt_sb[:, base : base + 257],
                start=(i == 0), stop=(i == NT - 1),
            )
            nc.tensor.matmul(
                S1, lhsT=xt_sb[:, base + 128 : base + 256],
                rhs=xt_sb[:, base : base + 257],
                start=(i == 0), stop=(i == NT - 1),
            )
        nc.scalar.copy(out=S0_sb, in_=S0)
        nc.vector.tensor_copy(S1_sb, S1)

        # uv = [u | v] = sx^T [phi_w^T | g_w^T]
        uvp = psC.tile([128, 256], f32, tag="c", name="uvp")
        nc.tensor.matmul(
            uvp[0:1, :], lhsT=S0_sb[:, 256:257], rhs=wpk16[:, 0:256],
            start=True, stop=False,
        )
        nc.tensor.matmul(
            uvp[0:1, :], lhsT=S1_sb[:, 256:257], rhs=wpk16[:, 256:512],
            start=False, stop=True,
        )
        nc.vector.tensor_copy(uv_sb, uvp[0:1, :])
        # row0 = u + N*bp (fp32 psum + fp32 aux -> fp16)
        nc.vector.tensor_add(row0_sb, uvp[0:1, 0:128], aux_sb[0:1, 256:384])

        # SG[c, d'] = sum_c2 S[c2, c] g_w[d', c2]  (S symmetric)
        SGp = psC.tile([128, 256], f32, tag="c", name="SGp")
        for h in range(2):
            hsl = slice(h * 128, (h + 1) * 128)
            nc.tensor.matmul(
                SGp[:, hsl], lhsT=S0_sb[:, hsl], rhs=wpk16[:, 128:256],
                start=True, stop=False,
            )
            nc.tensor.matmul(
                SGp[:, hsl], lhsT=S1_sb[:, hsl], rhs=wpk16[:, 384:512],
                start=False, stop=True,
            )
        nc.scalar.copy(out=SG_sb, in_=SGp)

        # Mt[d', d] = M[d, d'] = SG^T phi_w^T + bg(u+N bp)^T + v bp^T
        Mtp = psC.tile([128, 128], f32, tag="c", name="Mtp")
        nc.tensor.matmul(
            Mtp, lhsT=SG_sb[:, 0:128], rhs=wpk16[:, 0:128],
            start=True, stop=False,
        )
        nc.tensor.matmul(
            Mtp, lhsT=SG_sb[:, 128:256], rhs=wpk16[:, 256:384],
            start=False, stop=False,
        )
        nc.tensor.matmul(
            Mtp, lhsT=aux16[0:1, 0:128], rhs=row0_sb,
            start=False, stop=False,
        )
        nc.tensor.matmul(
            Mtp, lhsT=uv_sb[0:1, 128:256], rhs=aux16[0:1, 128:256],
            start=False, stop=True,
        )
        nc.scalar.copy(out=Mt_sb, in_=Mtp)

        # R1[d, c] = sum_d' Mt[d', d] w''[c, d']  (= V[c, d])
        R1p = psC.tile([128, 256], f32, tag="c", name="R1p")
        nc.tensor.matmul(R1p, lhsT=Mt_sb, rhs=wpk16[:, 512:768], start=True, stop=True)
        nc.scalar.copy(out=R1_sb, in_=R1p)

        # T2 halves: T2[c', c] = theta_w^T R1 + I  (identity via Ipk matmul)
        for h in range(2):
            T2p = psC.tile([128, 256], f32, tag="c", name=f"T2p{h}")
            nc.tensor.matmul(
                T2p, lhsT=wpk16[:, 768 + 128 * h : 896 + 128 * h], rhs=R1_sb,
                start=True, stop=False,
            )
            nc.tensor.matmul(
                T2p, lhsT=wpk16[:, 1024:1152],
                rhs=wpk16[:, 1024 + 256 * h : 1280 + 256 * h],
                start=False, stop=True,
            )
            if h == 0:
                nc.scalar.copy(out=T2_sb[:, 0:256], in_=T2p)
            else:
                nc.vector.tensor_copy(T2_sb[:, 256:512], T2p)

        # b2[c] = sum_d R1[d, c] bt[d] + b'[c]
        b2p = psC.tile([128, 2], f32, tag="c", name="b2p")
        for h in range(2):
            nc.tensor.matmul(
                b2p[:, h : h + 1], lhsT=R1_sb[:, h * 128 : (h + 1) * 128],
                rhs=wpk16[:, 1536:1537], start=True, stop=False,
            )
            nc.tensor.matmul(
                b2p[:, h : h + 1], lhsT=aux16[0:1, 256 + 128 * h : 384 + 128 * h],
                rhs=wpk16[0:1, 1024:1025], start=False, stop=True,
            )
        nc.vector.tensor_copy(b2_sb, b2p)

        # z[c, n] = sum_c' T2[c', c] X[c', n] + b2[c]; fp16 out
        engs = [nc.scalar, nc.vector]
        k = 0
        for j in range(6):
            for hc in range(2):
                pz = psZ.tile([128, 512], f32, tag="z", name=f"pz{j}_{hc}")
                nc.tensor.matmul(
                    pz, lhsT=T2_sb[:, 128 * hc : 128 * hc + 128],
                    rhs=xc_sb[:, j * 1024 : j * 1024 + 512],
                    start=True, stop=False,
                )
                nc.tensor.matmul(
                    pz, lhsT=T2_sb[:, 256 + 128 * hc : 384 + 128 * hc],
                    rhs=xc_sb[:, j * 1024 + 512 : (j + 1) * 1024],
                    start=False, stop=True,
                )
                z_sb = zpool.tile([128, 512], f16, tag="zs", name=f"z{j}_{hc}")
                eng = engs[k % 2]
                k += 1
                if eng is nc.scalar:
                    nc.scalar.activation(
                        out=z_sb, in_=pz, func=AF.Identity,
                        bias=b2_sb[:, hc : hc + 1], scale=1.0,
                    )
                    dma_eng = nc.scalar
                else:
                    eng.tensor_scalar_add(z_sb, pz, b2_sb[:, hc : hc + 1])
                    dma_eng = nc.sync
                dma_eng.dma_start(
                    out=out[:, hc * 3072 + j * 512 : hc * 3072 + (j + 1) * 512],
                    in_=z_sb,
                )

    nc.compile()
    return nc


def _get_nc():
    global _NC
    if _NC is None:
        _NC = _build_nc()
    return _NC


# test.py reads this after a traced run to get exec_time_ns
last_results = None


def _prep_inputs(inputs):
    f16 = np.float16

    x = np.asarray(inputs["x"], dtype=np.float32)
    theta_w = np.asarray(inputs["theta_w"], np.float32)
    theta_b = np.asarray(inputs["theta_b"], np.float32)
    phi_w = np.asarray(inputs["phi_w"], np.float32)
    phi_b = np.asarray(inputs["phi_b"], np.float32)
    g_w = np.asarray(inputs["g_w"], np.float32)
    g_b = np.asarray(inputs["g_b"], np.float32)
    w_w = np.asarray(inputs["w_w"], np.float32)
    w_b = np.asarray(inputs["w_b"], np.float32)
    bn_gamma = np.asarray(inputs["bn_gamma"], np.float32)
    bn_beta = np.asarray(inputs["bn_beta"], np.float32)
    bn_mean = np.asarray(inputs["bn_mean"], np.float32)
    bn_var = np.asarray(inputs["bn_var"], np.float32)

    inv = bn_gamma / np.sqrt(bn_var + BN_EPS)
    bprime = inv * (w_b - bn_mean) + bn_beta                  # (C,)
    wpp = (w_w * inv[:, None]) / N                            # w'' (C, D)

    # wpk: per-partition packed weights (f16 cols)
    pgWT = np.concatenate(
        [phi_w.T[0:128], g_w.T[0:128], phi_w.T[128:256], g_w.T[128:256]],
        axis=1,
    )                                                         # (128, 512)
    Ipk = np.zeros((128, 512), np.float32)
    Ipk[np.arange(128), np.arange(128)] = 1.0                 # [I | 0]
    Ipk[np.arange(128), 384 + np.arange(128)] = 1.0           # [0 | I]
    btc = np.zeros((128, 4), np.float32)
    btc[:, 0] = theta_b
    wpk_f16 = np.concatenate(
        [pgWT, wpp.T, theta_w, Ipk, btc], axis=1
    ).astype(f16)                                             # (128, 1540)
    assert wpk_f16.shape == (128, 1540), wpk_f16.shape
    wpk = np.ascontiguousarray(wpk_f16).view(np.float32)      # (128, 770)

    aux_f16 = np.concatenate(
        [g_b, phi_b, bprime]
    ).astype(f16)                                             # 512 f16 = 1024 B
    aux_f32 = (N * phi_b).astype(np.float32)                  # 512 B
    aux = np.concatenate(
        [aux_f16.view(np.uint8), aux_f32.view(np.uint8)]
    ).view(np.float32)[None, :]                               # (1, 384)

    x16 = x.reshape(B, C, N).astype(f16)
    xt = np.ones((B, NT, 128, 257), f16)
    xt[:, :, :, 0:256] = x16.transpose(0, 2, 1).reshape(B, NT, 128, C)
    xt = xt.reshape(B, 128 * NT, 257)  # will re-chunk below
    xt = np.ascontiguousarray(
        xt.reshape(B, NT, 128, 257).transpose(0, 2, 1, 3).reshape(B, 128, NT * 257)
    )
    xc = np.ascontiguousarray(
        x16.reshape(B, 2, 128, 6, 512).transpose(0, 2, 3, 1, 4).reshape(B, 128, 6144)
    )
    return xt, xc, {"wpk": wpk, "aux": aux}


def kernel(**inputs):
    from concourse.bass_utils import run_bass_kernel_spmd

    global last_results

    xt, xc, shared = _prep_inputs(inputs)
    in_maps = [
        dict(shared, xt=np.ascontiguousarray(xt[b]), xc=np.ascontiguousarray(xc[b]))
        for b in range(B)
    ]

    nc = _get_nc()
    res = run_bass_kernel_spmd(nc, in_maps, list(range(N_CORES)))
    last_results = res

    outs = np.stack([res.results[b]["out"] for b in range(B)])  # (B, 128, 6144)
    z = outs.reshape(B, 128, 2, 3072).transpose(0, 2, 1, 3).reshape(B, C, N)
    return z.reshape(B, C, HH, WW).astype(np.float32)


# revision 13
# speedup vs baseline: 1.0987x; 1.0575x over previous
"""Non-local (dot-product attention) block kernel for Trainium2, 8 cores.

Reference math (per sample):
    t = theta_w @ X + bt 1^T            (D, N)
    p = phi_w @ X + bp 1^T              (D, N)
    g = (g_w @ X + bg 1^T)^T            (N, D)
    f = t^T p / N;  y = f g;  z = BN(w_w y^T) + x

Gram-form collapse (no softmax => pure matmul associativity). Since phi/g
only appear through M = p g, and t only through W2 = V theta_w:
    S  = X X^T                      (C, C)  Gram matrix
    sx = X 1                        (C,)    row sums (free via ones column)
    Mt = g_w S phi_w^T + bg(u+N bp)^T + v bp^T   u=phi_w sx, v=g_w sx
    R1 = Mt^T w''^T  (= V^T)        w'' = diag(inv) w_w / N
    T2 = theta_w^T R1 + I           (= W2^T + I: residual folded into weights)
    b2 = R1^T bt + b'
    z  = T2^T X + b2 1^T            one (C,C) x (C,N) output matmul

So the per-pixel work is just two C x C x N matmuls (Gram + output); the
whole projection/bias machinery shrinks to a ~10-matmul D-sized chain.

Device plan per core (data-parallel, one sample per core, no collectives):
  - Inputs: x uploaded twice (n-major xt for the Gram, c-major xc for the
    output matmul), fp16; weights byte-packed into wpk + a 1-partition aux.
  - S accumulates in PSUM over 24 pixel chunks streamed by DMA; a host-side
    ones column (257th) makes column 256 of S the row-sums sx.
  - Chain runs on tiny matmuls with ACT/DVE/Pool doing the PSUM->SBUF hops.
  - Output phase: 12 (128,512) matmul pairs; per-partition bias b2 applied
    during the PSUM->SBUF copy (rotated over ACT/DVE/Pool), fp16 out DMA
    issued by the same engine that assembled the tile; host widens to fp32.
"""

import numpy as np

B, C, HH, WW = 8, 256, 96, 32
N = HH * WW          # 3072
D = 128              # inter_channels
BN_EPS = 1e-5
NT = N // 128        # 24 pixel chunks
N_CORES = 8
NWARM = 4            # dummy matmuls to lift the PE HAM clock gate

_NC = None


def _build_nc():
    from contextlib import ExitStack

    import concourse.bass as bass
    import concourse.bacc as bacc
    import concourse.tile as tile
    from concourse import mybir

    f32 = mybir.dt.float32
    f16 = mybir.dt.float16
    AF = mybir.ActivationFunctionType

    nc = bacc.Bacc(
        "TRN2",
        target_bir_lowering=False,
        debug=False,
        num_devices=N_CORES,
    )

    # xt: 24 blocks of 257 cols: block i = x^T[128i:128(i+1), :] | ones
    xt = nc.dram_tensor("xt", [128, NT * 257], f16, kind="ExternalInput").ap()
    # xc: 6 blocks of 1024: block j = [x[0:128, 512j:512j+512] | x[128:256, ...]]
    xc = nc.dram_tensor("xc", [128, 6144], f16, kind="ExternalInput").ap()
    # wpk f16 cols: pgWT 512 | wT2 256 | thW 256 | Ipk 512 | bt 2 | pad 2
    wpk = nc.dram_tensor("wpk", [128, 770], f32, kind="ExternalInput").ap()
    # aux f16 cols: bg 128 | bp 128 | b'0 128 | b'1 128 ; f32 cols 256:384 = N*bp
    aux = nc.dram_tensor("aux", [1, 384], f32, kind="ExternalInput").ap()
    # out: [z[0:128, :] | z[128:256, :]] fp16
    out = nc.dram_tensor("out", [128, 6144], f16, kind="ExternalOutput").ap()

    with tile.TileContext(nc) as tc, ExitStack() as ctx:
        const = ctx.enter_context(tc.tile_pool(name="const", bufs=1))
        zpool = ctx.enter_context(tc.tile_pool(name="zpool", bufs=4))
        psS = ctx.enter_context(tc.tile_pool(name="psS", bufs=1, space="PSUM"))
        psC = ctx.enter_context(tc.tile_pool(name="psC", bufs=2, space="PSUM"))
        psZ = ctx.enter_context(tc.tile_pool(name="psZ", bufs=3, space="PSUM"))

        xt_sb = const.tile([128, NT * 257], f16)
        xc_sb = const.tile([128, 6144], f16)
        wpk_sb = const.tile([128, 770], f32)
        aux_sb = const.tile([1, 384], f32)
        S0_sb = const.tile([128, 257], f16)
        S1_sb = const.tile([128, 257], f16)
        uv_sb = const.tile([1, 256], f16)
        row0_sb = const.tile([1, 128], f16)
        SG_sb = const.tile([128, 256], f16)
        Mt_sb = const.tile([128, 128], f16)
        R1_sb = const.tile([128, 256], f16)
        T2_sb = const.tile([128, 512], f16)
        b2_sb = const.tile([128, 2], f32)
        wz = const.tile([128, 512], f16)

        wpk16 = wpk_sb.bitcast(f16)   # (128, 1540)
        aux16 = aux_sb.bitcast(f16)   # (1, 768)

        # Each dma_start costs ~610ns on its issuing sequencer and rings
        # process their FIFO in order, so: xt pieces own the sync ring (the
        # S phase streams them), xc follows on the same ring (needed only at
        # the output phase), wpk/aux ride the otherwise-idle scalar ring.
        nc.scalar.dma_start(out=wpk_sb, in_=wpk)
        nc.scalar.dma_start(out=aux_sb, in_=aux)
        for p in range(6):
            psl = slice(p * 4 * 257, (p + 1) * 4 * 257)
            nc.sync.dma_start(out=xt_sb[:, psl], in_=xt[:, psl])
        for q in range(2):
            qsl = slice(q * 3072, (q + 1) * 3072)
            nc.sync.dma_start(out=xc_sb[:, qsl], in_=xc[:, qsl])

        # PE warm-up on a zeroed tile while the first xt piece is in flight
        # (HAM clock gate needs ~3.4us of sustained PE activity to lift
        # 1.2 -> 2.4 GHz; the S matmuls continue the streak).
        nc.vector.memset(wz, 0.0)
        for _ in range(NWARM):
            wup = psZ.tile([128, 512], f32, tag="z", name="wup")
            nc.tensor.matmul(wup, lhsT=wz[:, 0:128], rhs=wz, start=True, stop=True)

        # S = X X^T accumulated over 24 pixel chunks; col 256 = sx (ones col)
        S0 = psS.tile([128, 512], f32, tag="s0", name="S0")[:, 0:257]
        S1 = psS.tile([128, 512], f32, tag="s1", name="S1")[:, 0:257]
        for i in range(NT):
            base = i * 257
            nc.tensor.matmul(
                S0, lhsT=xt_sb[:, base : base + 128],
                rhs=xt_sb[:, base : base + 257],
                start=(i == 0), stop=(i == NT - 1),
            )
            nc.tensor.matmul(
                S1, lhsT=xt_sb[:, base + 128 : base + 256],
                rhs=xt_sb[:, base : base + 257],
                start=(i == 0), stop=(i == NT - 1),
            )
        nc.scalar.copy(out=S0_sb, in_=S0)
        nc.vector.tensor_copy(S1_sb, S1)

        # uv = [u | v] = sx^T [phi_w^T | g_w^T]
        uvp = psC.tile([128, 256], f32, tag="c", name="uvp")
        nc.tensor.matmul(
            uvp[0:1, :], lhsT=S0_sb[:, 256:257], rhs=wpk16[:, 0:256],
            start=True, stop=False,
        )
        nc.tensor.matmul(
            uvp[0:1, :], lhsT=S1_sb[:, 256:257], rhs=wpk16[:, 256:512],
            start=False, stop=True,
        )
        nc.vector.tensor_copy(uv_sb, uvp[0:1, :])
        # row0 = u + N*bp (fp32 psum + fp32 aux -> fp16)
        nc.vector.tensor_add(row0_sb, uvp[0:1, 0:128], aux_sb[0:1, 256:384])

        # SG[c, d'] = sum_c2 S[c2, c] g_w[d', c2]  (S symmetric)
        SGp = psC.tile([128, 256], f32, tag="c", name="SGp")
        for h in range(2):
            hsl = slice(h * 128, (h + 1) * 128)
            nc.tensor.matmul(
                SGp[:, hsl], lhsT=S0_sb[:, hsl], rhs=wpk16[:, 128:256],
                start=True, stop=False,
            )
            nc.tensor.matmul(
                SGp[:, hsl], lhsT=S1_sb[:, hsl], rhs=wpk16[:, 384:512],
                start=False, stop=True,
            )
        nc.scalar.copy(out=SG_sb, in_=SGp)

        # Mt[d', d] = M[d, d'] = SG^T phi_w^T + bg(u+N bp)^T + v bp^T
        Mtp = psC.tile([128, 128], f32, tag="c", name="Mtp")
        nc.tensor.matmul(
            Mtp, lhsT=SG_sb[:, 0:128], rhs=wpk16[:, 0:128],
            start=True, stop=False,
        )
        nc.tensor.matmul(
            Mtp, lhsT=SG_sb[:, 128:256], rhs=wpk16[:, 256:384],
            start=False, stop=False,
        )
        nc.tensor.matmul(
            Mtp, lhsT=aux16[0:1, 0:128], rhs=row0_sb,
            start=False, stop=False,
        )
        nc.tensor.matmul(
            Mtp, lhsT=uv_sb[0:1, 128:256], rhs=aux16[0:1, 128:256],
            start=False, stop=True,
        )
        nc.scalar.copy(out=Mt_sb, in_=Mtp)

        # R1[d, c] = sum_d' Mt[d', d] w''[c, d']  (= V[c, d])
        R1p = psC.tile([128, 256], f32, tag="c", name="R1p")
        nc.tensor.matmul(R1p, lhsT=Mt_sb, rhs=wpk16[:, 512:768], start=True, stop=True)
        nc.scalar.copy(out=R1_sb, in_=R1p)

        # T2 halves: T2[c', c] = theta_w^T R1 + I  (identity via Ipk matmul)
        for h in range(2):
            T2p = psC.tile([128, 256], f32, tag="c", name=f"T2p{h}")
            nc.tensor.matmul(
                T2p, lhsT=wpk16[:, 768 + 128 * h : 896 + 128 * h], rhs=R1_sb,
                start=True, stop=False,
            )
            nc.tensor.matmul(
                T2p, lhsT=wpk16[:, 1024:1152],
                rhs=wpk16[:, 1024 + 256 * h : 1280 + 256 * h],
                start=False, stop=True,
            )
            if h == 0:
                nc.scalar.copy(out=T2_sb[:, 0:256], in_=T2p)
            else:
                nc.vector.tensor_copy(T2_sb[:, 256:512], T2p)

        # b2[c] = sum_d R1[d, c] bt[d] + b'[c]
        b2p = psC.tile([128, 2], f32, tag="c", name="b2p")
        for h in range(2):
            nc.tensor.matmul(
                b2p[:, h : h + 1], lhsT=R1_sb[:, h * 128 : (h + 1) * 128],
                rhs=wpk16[:, 1536:1537], start=True, stop=False,
            )
            nc.tensor.matmul(
                b2p[:, h : h + 1], lhsT=aux16[0:1, 256 + 128 * h : 384 + 128 * h],
                rhs=wpk16[0:1, 1024:1025], start=False, stop=True,
            )
        nc.vector.tensor_copy(b2_sb, b2p)

        # z[c, n] = sum_c' T2[c', c] X[c', n] + b2[c]; fp16 out.
        # Per 512-pixel chunk j: two PSUM tiles (one per c-half), assembled
        # in parallel by ACT (half 0, bias via activation) and DVE (half 1,
        # tensor_scalar), one gpsimd-issued DMA covering both c-halves of
        # the chunk via a strided DRAM AP.
        for j in range(6):
            z_sb = zpool.tile([128, 1024], f16, tag="zs", name=f"z{j}")
            for hc in range(2):
                pz = psZ.tile([128, 512], f32, tag="z", name=f"pz{j}_{hc}")
                nc.tensor.matmul(
                    pz, lhsT=T2_sb[:, 128 * hc : 128 * hc + 128],
                    rhs=xc_sb[:, j * 1024 : j * 1024 + 512],
                    start=True, stop=False,
                )
                nc.tensor.matmul(
                    pz, lhsT=T2_sb[:, 256 + 128 * hc : 384 + 128 * hc],
                    rhs=xc_sb[:, j * 1024 + 512 : (j + 1) * 1024],
                    start=False, stop=True,
                )
                if hc == 0:
                    nc.scalar.activation(
                        out=z_sb[:, 0:512], in_=pz, func=AF.Identity,
                        bias=b2_sb[:, 0:1], scale=1.0,
                    )
                else:
                    nc.vector.tensor_scalar_add(
                        z_sb[:, 512:1024], pz, b2_sb[:, 1:2]
                    )
            out_ap = bass.AP(
                tensor=out.tensor, offset=j * 512,
                ap=[[6144, 128], [3072, 2], [1, 512]],
            )
            nc.gpsimd.dma_start(out=out_ap, in_=z_sb)

    nc.compile()
    return nc


def _get_nc():
    global _NC
    if _NC is None:
        _NC = _build_nc()
    return _NC


# test.py reads this after a traced run to get exec_time_ns
last_results = None


def _prep_inputs(inputs):
    f16 = np.float16

    x = np.asarray(inputs["x"], dtype=np.float32)
    theta_w = np.asarray(inputs["theta_w"], np.float32)
    theta_b = np.asarray(inputs["theta_b"], np.float32)
    phi_w = np.asarray(inputs["phi_w"], np.float32)
    phi_b = np.asarray(inputs["phi_b"], np.float32)
    g_w = np.asarray(inputs["g_w"], np.float32)
    g_b = np.asarray(inputs["g_b"], np.float32)
    w_w = np.asarray(inputs["w_w"], np.float32)
    w_b = np.asarray(inputs["w_b"], np.float32)
    bn_gamma = np.asarray(inputs["bn_gamma"], np.float32)
    bn_beta = np.asarray(inputs["bn_beta"], np.float32)
    bn_mean = np.asarray(inputs["bn_mean"], np.float32)
    bn_var = np.asarray(inputs["bn_var"], np.float32)

    inv = bn_gamma / np.sqrt(bn_var + BN_EPS)
    bprime = inv * (w_b - bn_mean) + bn_beta                  # (C,)
    wpp = (w_w * inv[:, None]) / N                            # w'' (C, D)

    # wpk: per-partition packed weights (f16 cols)
    pgWT = np.concatenate(
        [phi_w.T[0:128], g_w.T[0:128], phi_w.T[128:256], g_w.T[128:256]],
        axis=1,
    )                                                         # (128, 512)
    Ipk = np.zeros((128, 512), np.float32)
    Ipk[np.arange(128), np.arange(128)] = 1.0                 # [I | 0]
    Ipk[np.arange(128), 384 + np.arange(128)] = 1.0           # [0 | I]
    btc = np.zeros((128, 4), np.float32)
    btc[:, 0] = theta_b
    wpk_f16 = np.concatenate(
        [pgWT, wpp.T, theta_w, Ipk, btc], axis=1
    ).astype(f16)                                             # (128, 1540)
    assert wpk_f16.shape == (128, 1540), wpk_f16.shape
    wpk = np.ascontiguousarray(wpk_f16).view(np.float32)      # (128, 770)

    aux_f16 = np.concatenate(
        [g_b, phi_b, bprime]
    ).astype(f16)                                             # 512 f16 = 1024 B
    aux_f32 = (N * phi_b).astype(np.float32)                  # 512 B
    aux = np.concatenate(
        [aux_f16.view(np.uint8), aux_f32.view(np.uint8)]
    ).view(np.float32)[None, :]                               # (1, 384)

    x16 = x.reshape(B, C, N).astype(f16)
    xt = np.ones((B, NT, 128, 257), f16)
    xt[:, :, :, 0:256] = x16.transpose(0, 2, 1).reshape(B, NT, 128, C)
    xt = xt.reshape(B, 128 * NT, 257)  # will re-chunk below
    xt = np.ascontiguousarray(
        xt.reshape(B, NT, 128, 257).transpose(0, 2, 1, 3).reshape(B, 128, NT * 257)
    )
    xc = np.ascontiguousarray(
        x16.reshape(B, 2, 128, 6, 512).transpose(0, 2, 3, 1, 4).reshape(B, 128, 6144)
    )
    return xt, xc, {"wpk": wpk, "aux": aux}


def kernel(**inputs):
    from concourse.bass_utils import run_bass_kernel_spmd

    global last_results

    xt, xc, shared = _prep_inputs(inputs)
    in_maps = [
        dict(shared, xt=np.ascontiguousarray(xt[b]), xc=np.ascontiguousarray(xc[b]))
        for b in range(B)
    ]

    nc = _get_nc()
    res = run_bass_kernel_spmd(nc, in_maps, list(range(N_CORES)))
    last_results = res

    outs = np.stack([res.results[b]["out"] for b in range(B)])  # (B, 128, 6144)
    z = outs.reshape(B, 128, 2, 3072).transpose(0, 2, 1, 3).reshape(B, C, N)
    return z.reshape(B, C, HH, WW).astype(np.float32)
